# revision 71
# baseline (speedup 1.0000x reference)
"""Trainium2 Bass kernel for CornerBoundingBoxEMDLoss.

For each sample: 8x8 pairwise corner distances, then exact min-cost perfect
matching via meet-in-the-middle:

  min over perms = min over 70 4-subsets T of
      (min assignment of preds {0,1,2,3} onto T)
    + (min assignment of preds {4,5,6,7} onto complement(T))

computed hierarchically: pred pairs -> target pairs (L1, one-hot GEMM with
two orderings + elementwise min), pairs -> quads (L2, one-hot GEMM over the
6 = C(4,2) pair-to-half assignments per 2+2 split + group-min), then a
fused add+min over the 70 complement-aligned A+B sums. Exact same minimum
as brute force over 8! permutations, ~50x less arithmetic.

Data-parallel across 8 NeuronCores: 512 samples per core, as 4 chunks of
128 samples (samples on SBUF partitions). Performance notes (measured on
HW; all deltas are same-session A/B against the previous kernel):

- fp16 everywhere off-PSUM: 2-byte packed SBUF operands put DVE
  tensor_tensor ops in their 2x mode (~0.55 ns/elem vs 1.04); fp16's
  10-bit mantissa keeps rel err ~1e-3 (4x better than the old bf16 path).
  tensor_reduce has NO fast mode (~1.19 ns/elem regardless of dtype or
  space), so the min-over-6 reduces read PSUM directly - copies to sbuf
  buy nothing for reduces.
- Compact fp16 input (48 cols/chunk, half the baseline's DMA bytes); the
  (i,j) broadcast happens inside the DVE sub's access patterns. A
  pre-broadcast host layout was 4x the bytes and moved the data-landing
  time ~1.6us later - DMA bytes, not DVE ops, gate the front.
- Data split across the sync+scalar HWDGE queues so both pairs land
  ~9.5us. (Pool-SWDGE data wedged the DVE queue; an all-sync chain
  serialized triggers. The scalar-queue DMA costs a second act-table
  load, but both loads hide inside the DMA-wait window.) The transpose
  identity is built on-device by idle Pool (memset + affine_select)
  instead of a fourth DMA: the 8 cores' simultaneous input DMAs contend
  for HBM in bad reps (descriptors stretch 6ns -> 45-180ns).
- Each chunk's A|B min-over-6 is ONE grouped DVE reduce over a regular
  4-dim AP spanning the two psum banks of a bank-padded [128,1024] tile
  (matmul outputs may not straddle banks). Chunk 0 instead splits A into
  its own tile so the first reduce starts after the first (cold) GEMM,
  and fuses its L1 ordering-min as a strided-inner psum reduce (no ACT
  copy) - both shorten the pipeline-fill chain that gates the crunch.
- Per-chunk loss: Pool adds A+B (sbuf-only; Pool cannot touch PSUM and
  has no min kernel), DVE does a [70]->[1] reduce into loss[:, c]; the
  last chunk adds on DVE to skip the pool round-trip on the tail.
- L1 matmuls for chunk c+1 are emitted before chunk c's L2 matmuls so
  the in-order PE queue doesn't starve the next chunk's min chain.
"""

import itertools

import numpy as np
import ml_dtypes

import concourse.bacc as bacc
import concourse.mybir as mybir
import concourse.tile as tile

N_CORES = 8
B_TOTAL = 4096
B_CORE = B_TOTAL // N_CORES          # 512
N_CHUNKS = 4
CHUNK = B_CORE // N_CHUNKS           # 128

F32 = mybir.dt.float32
F16 = mybir.dt.float16
FP8 = mybir.dt.float8e4


def _build_constants():
    """Packed one-hot selection matrices.

    cpack [128, 1064] fp8e4m3 (one-hot -> exact):
      cols   0:112  l1 ordering 0   (partitions 0:64 and replicated 64:128)
      cols 112:224  l1 ordering 1   (same replication)
      cols 224:1064 l2 (partitions 0:112): 840 = [A-side 70*6 | B-side 70*6]
    """
    pairs = list(itertools.combinations(range(8), 2))            # 28
    pair_idx = {p: i for i, p in enumerate(pairs)}
    subs4 = list(itertools.combinations(range(8), 4))            # 70
    pred_pairs = [(0, 1), (2, 3), (4, 5), (6, 7)]

    l1o0 = np.zeros((64, 112), dtype=np.float32)
    l1o1 = np.zeros((64, 112), dtype=np.float32)
    for q, (i0, i1) in enumerate(pred_pairs):
        for p, (a, b) in enumerate(pairs):
            col = q * 28 + p
            l1o0[i0 * 8 + a, col] = 1; l1o0[i1 * 8 + b, col] = 1
            l1o1[i0 * 8 + b, col] = 1; l1o1[i1 * 8 + a, col] = 1

    # all 6 C(T,2) choices of which target pair the first pred pair gets
    # (each 2+2 split appears twice with the pair roles swapped -- those are
    # distinct matchings, both needed)
    l2 = np.zeros((112, 840), dtype=np.float32)
    for t, T in enumerate(subs4):
        for s, S in enumerate(itertools.combinations(T, 2)):
            R = tuple(sorted(set(T) - set(S)))
            l2[0 * 28 + pair_idx[S], t * 6 + s] = 1
            l2[1 * 28 + pair_idx[R], t * 6 + s] = 1
        TB = tuple(sorted(set(range(8)) - set(T)))               # complement
        for s, S in enumerate(itertools.combinations(TB, 2)):
            R = tuple(sorted(set(TB) - set(S)))
            l2[2 * 28 + pair_idx[S], 420 + t * 6 + s] = 1
            l2[3 * 28 + pair_idx[R], 420 + t * 6 + s] = 1

    cpack = np.zeros((128, 1064), dtype=np.float32)
    cpack[0:64, 0:112] = l1o0
    cpack[0:64, 112:224] = l1o1
    cpack[64:128, 0:224] = cpack[0:64, 0:224]
    cpack[0:112, 224:1064] = l2
    return cpack.astype(ml_dtypes.float8_e4m3)


class _Bacc(bacc.Bacc):
    """Bacc whose act-table chooser is steered to a single table.

    The stock chooser maps each activation func to the first table
    containing it (Square/Copy -> table 0, Sqrt -> table 3), emitting two
    back-to-back 1.3us ACT_TABLE_LOADs at stream head.  Table 3
    (sqrt_and_others) physically contains sqrt, square AND copy, so
    pruning those funcs from every other table's advertised set forces
    the chooser to table 3 and a single load.  Table ids still index the
    canonical act_info.json list, so the emitted NEFF is unchanged apart
    from dropping the redundant load.
    """

    def insert_act_table_loads(self):
        import bass_rust as _br
        from concourse.hw_specs import get_activation_tables

        has_activation = any(
            isinstance(i, mybir.InstActivation)
            for b in self.main_func.blocks
            for i in b.instructions
        )
        if not has_activation:
            return
        A = mybir.ActivationFunctionType
        prune = {A.Sqrt, A.Square, A.Copy}
        tables = []
        for name, funcs in get_activation_tables(self.m.arch).items():
            if name != "sqrt_and_others":
                funcs = set(funcs) - prune
            tables.append((name, set(funcs)))
        _br.insert_act_table_loads(self, tables)


def build_nc():
    nc = _Bacc("TRN2", target_bir_lowering=False, debug=False)

    # dataA: chunks 0,1; dataB: chunks 2,3. Per chunk a compact 48-col fp16
    # block [pred (i,c) 24 | targ (j,c) 24]; the (i,j)-broadcast happens in
    # the DVE sub's access patterns (4x fewer DMA bytes than pre-broadcast).
    dataA_d = nc.dram_tensor("dataA", [CHUNK, 96], F16, kind="ExternalInput")
    dataB_d = nc.dram_tensor("dataB", [CHUNK, 96], F16, kind="ExternalInput")
    cpack_d = nc.dram_tensor("cpack", [128, 1064], FP8, kind="ExternalInput")
    out_d = nc.dram_tensor("out", [CHUNK, N_CHUNKS], F32, kind="ExternalOutput")

    with tile.TileContext(nc) as tc:
        with (
            tc.tile_pool(name="consts", bufs=1) as cpool,
            tc.tile_pool(name="persist", bufs=1) as ppool,
            tc.tile_pool(name="work", bufs=3) as wpool,
            tc.tile_pool(name="pairs", bufs=2) as qpool,
            tc.tile_pool(name="psum_t", bufs=1, space="PSUM") as pst,
            tc.tile_pool(name="psum_a", bufs=2, space="PSUM") as psa,
            tc.tile_pool(name="psum_2", bufs=2, space="PSUM") as ps2p,
            tc.tile_pool(name="psum_20", bufs=1, space="PSUM") as ps20p,
        ):
            dataA = cpool.tile([CHUNK, 96], F16, tag="dataA")
            dataB = cpool.tile([CHUNK, 96], F16, tag="dataB")
            cpk = cpool.tile([128, 1064], FP8, tag="cpack")
            c_id = cpool.tile([128, 128], F16, tag="ident")
            # data split across the two HWDGE queues so both pairs land
            # ~together at ~9.5us; cpack rides second on sync. The
            # scalar-queue DMA costs a second act-table load but both loads
            # hide inside the DMA-wait window. The transpose identity is
            # built on-device by idle Pool (fewer DMA descriptors: the 8
            # cores' simultaneous input DMAs contend for HBM in bad reps).
            nc.sync.dma_start(dataA[:, :], dataA_d[:, :])
            nc.scalar.dma_start(dataB[:, :], dataB_d[:, :])
            nc.sync.dma_start(cpk[:, :], cpack_d[:, :])
            nc.gpsimd.memset(c_id[:, :], 1.0)
            nc.gpsimd.affine_select(c_id[:, :], c_id[:, :], [[1, 128]],
                                    mybir.AluOpType.is_equal, 0.0,
                                    base=0, channel_multiplier=-1)

            # PE p-state warm-up: real PE work starts ~10.8us; a continuous
            # dummy-matmul chain through the input-DMA window ends right
            # before it, so the transposes/GEMMs hit a ramped clock. The
            # dump rides chunk 0's psum bank (WAW-ordered before its GEMM).
            wz = cpool.tile([128, 256], F16, tag="wz")
            nc.vector.memset(wz[:, :], 0.0)
            wdump = ps20p.tile([128, 420], F32, tag="ps2a")
            for _ in range(16):
                nc.tensor.matmul(wdump[:, 0:256], wz[:, 0:128], wz[:, 0:256],
                                 start=True, stop=True)

            m_t = ppool.tile([112, B_CORE], F16, tag="m")
            loss = ppool.tile([128, N_CHUNKS], F32, tag="loss")
            scr = ppool.tile([128, 4 * 70], F16, tag="scr")

            d2p = [None, None]
            dtp = [None, None]
            # diff/sq are deliberately SHARED between the two pairs: the
            # WAR hazard forces pair 1's phase 1 to run strictly after
            # pair 0's, which the priority system cannot - the scheduler
            # otherwise interleaves pair 1's (ready) adds into pair 0's
            # sem-latency gaps, delaying the first transpose ~0.4us.
            diff = ppool.tile([CHUNK, 384], F16, tag="diff")
            sq = ppool.tile([CHUNK, 384], F16, tag="sq")

            def phase1_pair(pair):
                """d^2 for both chunks of a pair: broadcast fp16 add (targ
                negated on host) + packed mult (DVE 2x) + grouped 3-sum ->
                d2p [128, 2*64]. All on DVE: both pairs finish before the
                min-reduce crunch starts, so this rides DVE's early slack."""
                src = dataA if pair == 0 else dataB
                d2p[pair] = qpool.tile([CHUNK, 128], F16, tag="d2p",
                                       name="d2p")
                with nc.allow_low_precision("fp16 distance pipeline; rel "
                                            "err ~1e-3, gate is 2e-2"):
                    for h in range(2):
                        dsl = src[:, 48 * h: 48 * h + 48]
                        p_b = (dsl[:, 0:24].rearrange("p (i c) -> p i c", c=3)
                               .unsqueeze(2).broadcast_to((CHUNK, 8, 8, 3)))
                        t_b = (dsl[:, 24:48].rearrange("p (j c) -> p j c", c=3)
                               .unsqueeze(1).broadcast_to((CHUNK, 8, 8, 3)))
                        dv = diff[:, 192 * h: 192 * h + 192].rearrange(
                            "p (i j c) -> p i j c", i=8, j=8)
                        nc.vector.tensor_tensor(dv, p_b, t_b,
                                                op=mybir.AluOpType.add)
                    nc.vector.tensor_tensor(sq[:, :], diff[:, :], diff[:, :],
                                            op=mybir.AluOpType.mult)
                    nc.vector.tensor_reduce(
                        d2p[pair][:, :],
                        sq[:, :].rearrange("p (g c) -> p g c", c=3),
                        axis=mybir.AxisListType.X, op=mybir.AluOpType.add)

            def transpose_pair(pair):
                """[128 samples, 2x64 d2] -> fp16 dist [2x64, 128 samples]."""
                tp = pst.tile([128, 128], F16, tag="tp")
                nc.tensor.transpose(tp[:, :], d2p[pair][:, :], c_id[:, :])
                dtp[pair] = qpool.tile([128, 128], F16, tag="dtp", name="dtp")
                nc.scalar.activation(dtp[pair][:, :], tp[:, :],
                                     mybir.ActivationFunctionType.Sqrt)

            def l1(c, fuse=False):
                """pred-pair x target-pair costs for chunk c -> m_t cols.
                fuse=True mins the two orderings with a strided-inner DVE
                reduce straight from PSUM (no ACT copy): slower on DVE but
                ~0.4us shorter chain - right for chunk 0, which gates the
                whole crunch."""
                pair, half = divmod(c, 2)
                hp = slice(64 * half, 64 * half + 64)
                rhs = dtp[pair][hp, :]
                ps01 = psa.tile([112, 256], F32, tag="ps01")
                nc.tensor.matmul(ps01[:, 0:128], cpk[hp, 0:112], rhs,
                                 start=True, stop=True)
                nc.tensor.matmul(ps01[:, 128:256], cpk[hp, 112:224], rhs,
                                 start=True, stop=True)
                with nc.allow_low_precision("pair costs in fp16"):
                    if fuse:
                        nc.vector.tensor_reduce(
                            m_t[:, CHUNK * c: CHUNK * (c + 1)],
                            ps01[:, :].rearrange("p (o x) -> p x o", o=2),
                            axis=mybir.AxisListType.X, op=mybir.AluOpType.min)
                    elif c >= 2:
                        # dense-crunch chunks: ACT (which has slack there)
                        # copies BOTH orderings so the DVE min runs in 2x
                        # fp16 mode (a psum operand would force 1x)
                        s01 = wpool.tile([112, 256], F16, tag="s01",
                                         name="s01")
                        nc.scalar.activation(s01[:, 0:128], ps01[:, 0:128],
                                             mybir.ActivationFunctionType.Copy)
                        nc.scalar.activation(s01[:, 128:256], ps01[:, 128:256],
                                             mybir.ActivationFunctionType.Copy)
                        nc.vector.tensor_tensor(
                            m_t[:, CHUNK * c: CHUNK * (c + 1)], s01[:, 0:128],
                            s01[:, 128:256], op=mybir.AluOpType.min)
                    else:
                        # HW: TensorTensor may read at most one psum input
                        s1 = wpool.tile([112, 128], F16, tag="s1", name="s1")
                        nc.scalar.activation(s1[:, :], ps01[:, 128:256],
                                             mybir.ActivationFunctionType.Copy)
                        nc.vector.tensor_tensor(
                            m_t[:, CHUNK * c: CHUNK * (c + 1)], ps01[:, 0:128],
                            s1[:, :], op=mybir.AluOpType.min)

            def l2_mm(c):
                """quad-cost GEMMs for chunk c into ONE bank-padded psum
                tile: A side at cols 0:420 (bank 0), B side at 512:932
                (bank 1) - a matmul output may not straddle a psum bank.
                Chunk 0 splits A into its own tile (shared with the PE
                warm-up dump) so its min-reduce starts right after the
                first GEMM, filling the DVE's fill-phase gap."""
                msl = m_t[:, CHUNK * c: CHUNK * (c + 1)]
                if c == 0:
                    ps2a = ps20p.tile([128, 420], F32, tag="ps2a", name="ps2a")
                    ps2b = ps2p.tile([128, 1024], F32, tag="ps2", name="ps2")
                    nc.tensor.matmul(ps2a[:, :], msl, cpk[0:112, 224:644],
                                     start=True, stop=True)
                    nc.tensor.matmul(ps2b[:, 0:420], msl, cpk[0:112, 644:1064],
                                     start=True, stop=True)
                    return (ps2a, ps2b)
                ps2 = ps2p.tile([128, 1024], F32, tag="ps2", name="ps2")
                nc.tensor.matmul(ps2[:, 0:420], msl, cpk[0:112, 224:644],
                                 start=True, stop=True)
                nc.tensor.matmul(ps2[:, 512:932], msl, cpk[0:112, 644:1064],
                                 start=True, stop=True)
                return ps2

            def minred(c, ps2, tail=False):
                """min over the 6 assignments for BOTH sides in one DVE
                grouped reduce straight from PSUM (regular 4-dim AP over
                the two banks), then A+B and a [70]->[1] loss reduce into
                loss[:, c]. Pool does the add except on the last chunk,
                where DVE keeps it local to shorten the tail."""
                tab = wpool.tile([128, 140], F16, tag="tab", name="tab")
                with nc.allow_low_precision("fp16 min-reduce"):
                    if isinstance(ps2, tuple):
                        ps2a, ps2b = ps2
                        nc.vector.tensor_reduce(
                            tab[:, 0:70],
                            ps2a[:, :].rearrange("p (t s) -> p t s", s=6),
                            axis=mybir.AxisListType.X, op=mybir.AluOpType.min)
                        nc.vector.tensor_reduce(
                            tab[:, 70:140],
                            ps2b[:, 0:420].rearrange("p (t s) -> p t s", s=6),
                            axis=mybir.AxisListType.X, op=mybir.AluOpType.min)
                    else:
                        pv = (ps2[:, :].rearrange("p (e x) -> p e x", e=2)
                              [:, :, 0:420].rearrange("p e (t s) -> p e t s",
                                                      s=6))
                        nc.vector.tensor_reduce(
                            tab[:, :].rearrange("p (e t) -> p e t", e=2),
                            pv, axis=mybir.AxisListType.X,
                            op=mybir.AluOpType.min)
                    sl = scr[:, 70 * c: 70 * c + 70]
                    if tail:
                        nc.vector.tensor_tensor(sl, tab[:, 0:70],
                                                tab[:, 70:140],
                                                op=mybir.AluOpType.add)
                    else:
                        nc.gpsimd.tensor_add(sl, tab[:, 0:70],
                                             tab[:, 70:140])
                    if c % 2 == 1:
                        # one grouped [2,70]->[2] reduce per chunk pair
                        nc.vector.tensor_reduce(
                            loss[:, c - 1:c + 1],
                            scr[:, 70 * c - 70: 70 * c + 70].rearrange(
                                "p (e x) -> p e x", e=2),
                            axis=mybir.AxisListType.X, op=mybir.AluOpType.min)

            # pipelined schedule (engine streams stay dependency-ordered);
            # pair 0's transpose+sqrt outrank pair 1's phase 1 so the
            # scheduler doesn't park the first chunk behind it.
            phase1_pair(0)
            with tc.high_priority():
                transpose_pair(0)
            l1(0, fuse=True)
            ps2_0 = l2_mm(0)
            phase1_pair(1)
            l1(1)
            transpose_pair(1)
            minred(0, ps2_0)
            ps2_1 = l2_mm(1)
            l1(2)
            minred(1, ps2_1)
            ps2_2 = l2_mm(2)
            l1(3)
            minred(2, ps2_2)
            ps2_3 = l2_mm(3)
            minred(3, ps2_3, tail=True)

            # loss[p, c] = loss of sample c*128+p; host reorders
            nc.sync.dma_start(out_d[:, :], loss[:, :])

    nc.compile()
    return nc


_NC = None


def _get_nc():
    global _NC
    if _NC is None:
        _NC = build_nc()
    return _NC


def _input_maps(pred_corners, target_corners):
    cpack = _build_constants()
    pred = np.ascontiguousarray(pred_corners, dtype=np.float32)
    targ = np.ascontiguousarray(target_corners, dtype=np.float32)
    in_maps = []
    for k in range(N_CORES):
        sl = slice(k * B_CORE, (k + 1) * B_CORE)
        # compact per-chunk blocks [pred (i,c) 24 | -targ (j,c) 24] fp16
        # (targets negated so the on-device diff is an ADD - Pool has an
        # Add kernel but no Sub-with-broadcast guarantees)
        pk = pred[sl].reshape(N_CHUNKS, CHUNK, 24)
        tk = targ[sl].reshape(N_CHUNKS, CHUNK, 24)
        blk = np.concatenate([pk, -tk], axis=2).astype(np.float16)  # [4,128,48]
        dataA = np.ascontiguousarray(
            blk[0:2].transpose(1, 0, 2).reshape(CHUNK, 96))
        dataB = np.ascontiguousarray(
            blk[2:4].transpose(1, 0, 2).reshape(CHUNK, 96))
        in_maps.append({"dataA": dataA, "dataB": dataB, "cpack": cpack})
    return in_maps


def _gather(results):
    outs = []
    for k in range(N_CORES):
        o = results[k]["out"].reshape(CHUNK, N_CHUNKS).astype(np.float32)
        outs.append(np.ascontiguousarray(o.T).reshape(B_CORE))
    return np.concatenate(outs)


def kernel(pred_corners: np.ndarray, target_corners: np.ndarray) -> np.ndarray:
    from concourse.bass_utils import run_bass_kernel_spmd

    nc = _get_nc()
    in_maps = _input_maps(pred_corners, target_corners)
    res = run_bass_kernel_spmd(nc, in_maps, core_ids=list(range(N_CORES)))
    return _gather(res.results)
